# revision 22
# baseline (speedup 1.0000x reference)
"""Per-pixel 9x9 dynamic convolution (KPN denoiser) on 8 Trainium2 cores.

out[h,w,c] = sum_{ki,kj} padded_img[h+ki, w+kj, c] * wt[h, w, ki*9+kj]

Sharding: host reflect-pads the image and shards H rows across 8 cores
(128 output rows + 8 halo rows per core); per-pixel kernels shard the
same way; no cross-core communication.

v4 design (per core, W processed in two 512-wide halves):
- All device inputs are fp16, cast and laid out on the HOST, so DMA
  streams exactly the bytes the engines read (no cast-DMA f32 reads,
  no on-chip repack):
  - the image arrives as one [128, 9*3*520] tile per half: partition p
    holds, for each (ki, c), the 520-wide padded row p+ki. 9x row
    duplication is baked in on the host (tap-row shifts cross partition
    bases, which compute engines cannot do).
  - the weights arrive as 9 per-ki tap-plane tiles [128, 9*520]: block
    kj holds wt[., w, ki, kj] pre-shifted by kj (x = w + kj) and
    zero-padded, so every DVE operand starts 4B-aligned with step 1.
- DVE: ONE fp16 tensor_mul per (half, ki) covering all 3 channels and
  all 9 kj taps (free dims [(c,3),(kj,9),(x,520)]; the image operand
  broadcasts over kj with a 0-stride dim) -> 2x_1P mode throughout.
- PE accumulates the 81 taps per channel into PSUM with
  identity-stationary matmuls; the kj realignment happens for free via
  the moving-operand offset (window x in [kj, kj+512)).
- ACT evacuates PSUM to a planar staging tile and issues the output
  store; all loads go on the sync HWDGE ring (SWDGE would contend with
  DVE's 2-port mode for SBUF).

Projected per-core busy: DVE ~133us (bottleneck), PE ~105us, DMA ~30MB
~85us, ACT ~4us.
"""

import numpy as np

import concourse.bass as bass
import concourse.bacc as bacc
import concourse.mybir as mybir
from concourse.bass import AP
from concourse.bass_utils import run_bass_kernel_spmd
from concourse.masks import make_identity
from concourse.tile import TileContext

K = 9
PAD = K // 2  # 4
H = 1024
W = 1024
C = 3
NCORES = 8
R = H // NCORES  # 128 rows per core
HALF = W // 2  # 512
BW = HALF + 2 * PAD  # 520: per-tap plane width (kj shift baked in)
FKI = K * BW  # 4680: free size of one per-ki weight tile
FIMG = K * C * BW  # 14040: free size of one per-half image tile

f32 = mybir.dt.float32
f16 = mybir.dt.float16
f8 = mybir.dt.float8e4

last_results = None  # stash for test harness introspection


def _sub_ap(base: AP, free_off: int, dims) -> AP:
    """Build a free-dim access pattern on `base` (a full-tile [P, F] AP):
    keep the partition dim, replace free dims with `dims` ([step, count]
    pairs, in elements) at element offset `free_off`."""
    ap_pairs = [list(p) for p in base.ap]
    part = ap_pairs[0]
    return AP(
        base.tensor,
        base.offset + free_off,
        [part] + [[int(s), int(n)] for s, n in dims],
    )


def build_v4(rows=R, trn="TRN2", n_reps=1, probe=0, pair_kj=False, ident_f8=False,
             dedup_ldw=True):
    """Per-core program. img [rows, 2*FIMG] f16, wt [rows, 2*K*FKI] f16,
    out [rows, 2*C*HALF] f32 (half-major, then channel-planar).

    probe: timing probes. 1 = skip PE/evac/store (DVE+DMA only);
    2 = skip DVE products (PE+DMA only).
    pair_kj: one N=1024 matmul per kj pair with a 0-stride PSUM out AP —
    REJECTED by walrus codegen ISA checks; kept for reference.
    ident_f8: fp8e4 identity stationary (exact 0/1, faster FWL load).
    dedup_ldw: the Tile scheduler emits one LDWEIGHTS per matmul; with an
    identical full-array stationary each reload serializes after the
    previous matmul's stream (row_grp conflict), costing ~53ns x 486 MMs.
    This pass drops wait-free LDWEIGHTS that reload the stationary the PE
    already holds (measured 266 -> ~216 ns/matmul)."""
    nc = bacc.Bacc(trn)
    img = nc.declare_dram_parameter("img", [rows, 2 * FIMG], f16, isOutput=False)
    wt = nc.declare_dram_parameter("wt", [rows, 2 * K * FKI], f16, isOutput=False)
    out = nc.declare_dram_parameter("out", [rows, 2 * C * HALF], f32, isOutput=True)

    with TileContext(nc) as tc:
        with (
            tc.tile_pool(name="singles", bufs=1) as singles,
            tc.tile_pool(name="imgp", bufs=2) as imgp,
            tc.tile_pool(name="wtp", bufs=4) as wtp,
            tc.tile_pool(name="prodp", bufs=3) as prodp,
            tc.tile_pool(name="outp", bufs=2) as outp,
            tc.tile_pool(name="psump", bufs=2, space="PSUM") as psump,
        ):
            ident = singles.tile([128, 128], f8 if ident_f8 else f16)
            make_identity(nc, ident[:])
            fake = None
            if probe == 6:  # decoupling probe: PE streams this, not prod
                fake = singles.tile([rows, C * FKI], f16)
                nc.vector.memset(fake[:], 0.5)

            def _do_half(half):
                imt = imgp.tile([rows, FIMG], f16, tag="im")
                nc.sync.dma_start(
                    out=imt[:], in_=img[:, half * FIMG : (half + 1) * FIMG]
                )
                wts = {}
                for ki in range(K):
                    wt_t = wtp.tile([rows, FKI], f16, tag="wt")
                    col0 = (half * K + ki) * FKI
                    nc.sync.dma_start(out=wt_t[:], in_=wt[:, col0 : col0 + FKI])
                    wts[ki] = wt_t

                ps = {}
                for c in range(C):
                    ps_t = psump.tile([rows, HALF], f32, tag=f"ps{c}")
                    ps[c] = ps_t

                for ki in range(K):
                    if probe == 2:
                        prod = imt  # PE-only probe: stream the image tile
                    else:
                        prod = prodp.tile([rows, C * FKI], f16, tag="prod")
                    # prod[c, kj, x] = img[p+ki, x, c] * wt_shift[kj][x]
                    if probe != 2:
                        nc.vector.tensor_mul(
                            _sub_ap(prod[:], 0, [[FKI, C], [BW, K], [1, BW]]),
                            _sub_ap(
                                imt[:], ki * C * BW, [[BW, C], [0, K], [1, BW]]
                            ),
                            _sub_ap(wts[ki][:], 0, [[0, C], [BW, K], [1, BW]]),
                        )
                    if probe == 1:
                        continue
                    if probe == 6:
                        prod = fake
                    for c in range(C):
                        for kj in range(K):
                            # window x in [kj, kj+512) realigns tap kj onto w
                            nc.tensor.matmul(
                                ps[c][:],
                                ident[:rows, :rows],
                                _sub_ap(
                                    prod[:], c * FKI + kj * BW + kj, [[1, HALF]]
                                ),
                                start=(ki == 0 and kj == 0),
                                stop=(ki == K - 1 and kj == K - 1),
                            )

                if probe != 1:
                    ostage = outp.tile([rows, C * HALF], f32, tag="ostage")
                    for c in range(C):
                        nc.scalar.copy(
                            out=ostage[:, c * HALF : (c + 1) * HALF],
                            in_=ps[c][:],
                        )
                    nc.scalar.dma_start(
                        out=out[:, half * C * HALF : (half + 1) * C * HALF],
                        in_=ostage[:],
                    )

            def _body():
                for half in range(2):
                    _do_half(half)

            if n_reps == 1:
                _body()
            elif n_reps < 0:  # unrolled, for the timeline simulator
                for _ in range(-n_reps):
                    _body()
            else:
                with tc.For_i(0, n_reps, 1):
                    _body()

    if dedup_ldw:
        # Runs after the Tile scheduler (which created the per-matmul
        # LDWEIGHTS) and before nc.compile() (which would move matmul
        # waits onto them). Only wait/update-free exact-duplicate reloads
        # within a block are dropped, so synchronization is untouched.
        for f in nc.m.functions:
            for b in f.blocks:
                prev_key = None
                keep = []
                for i in b.instructions:
                    if i.opcode == "Ldweights":
                        key = repr(i.ins[0])
                        si = i.sync_info
                        clean = not si or (not si.on_wait and not si.on_update)
                        if key == prev_key and clean:
                            continue
                        prev_key = key
                    keep.append(i)
                b.instructions[:] = keep

    nc.compile()
    return nc


def _prep_inputs(unet_out: np.ndarray, cnn_out: np.ndarray):
    padded = np.pad(
        unet_out, ((PAD, PAD), (PAD, PAD), (0, 0)), mode="reflect"
    ).astype(np.float16)
    padc = np.ascontiguousarray(padded.transpose(2, 0, 1))  # [C, H+8, W+8] f16

    # image: [p][half][ki][c][x], x = padded col half*HALF + x
    img_all = np.empty((H, 2, K, C, BW), np.float16)
    for half in range(2):
        for ki in range(K):
            img_all[:, half, ki] = padc[
                :, ki : ki + H, half * HALF : half * HALF + BW
            ].transpose(1, 0, 2)

    # weights: [p][half][ki][kj][x] with tap (ki,kj) shifted by kj and
    # zero-padded to width BW
    wt_all = np.zeros((H, 2, K, K, BW), np.float16)
    cnn4 = cnn_out.reshape(H, 2, HALF, K * K)
    for ki in range(K):
        for kj in range(K):
            wt_all[:, :, ki, kj, kj : kj + HALF] = cnn4[:, :, :, ki * K + kj]

    in_maps = []
    for i in range(NCORES):
        in_maps.append(
            {
                "img": np.ascontiguousarray(
                    img_all[i * R : (i + 1) * R].reshape(R, 2 * FIMG)
                ),
                "wt": np.ascontiguousarray(
                    wt_all[i * R : (i + 1) * R].reshape(R, 2 * K * FKI)
                ),
            }
        )
    return in_maps


_nc_cache = {}


def kernel(unet_out: np.ndarray, cnn_out: np.ndarray, _reps=1, _probe=0) -> np.ndarray:
    global last_results
    unet_out = np.asarray(unet_out, dtype=np.float32)
    cnn_out = np.asarray(cnn_out, dtype=np.float32)
    if (_reps, _probe) not in _nc_cache:
        _nc_cache[(_reps, _probe)] = build_v4(n_reps=_reps, probe=_probe)
    nc = _nc_cache[(_reps, _probe)]
    in_maps = _prep_inputs(unet_out, cnn_out)
    res = run_bass_kernel_spmd(nc, in_maps, list(range(NCORES)))
    last_results = res
    outs = [
        res.results[i]["out"].reshape(R, 2, C, HALF) for i in range(NCORES)
    ]
    full = np.concatenate(outs, axis=0)  # [H, 2, C, HALF]
    return np.ascontiguousarray(full.transpose(0, 1, 3, 2).reshape(H, W, C))


# revision 27
# speedup vs baseline: 1.2417x; 1.2417x over previous
"""Per-pixel 9x9 dynamic convolution (KPN denoiser) on 8 Trainium2 cores.

out[h,w,c] = sum_{ki,kj} padded_img[h+ki, w+kj, c] * wt[h, w, ki*9+kj]

Sharding: host reflect-pads the image and shards H rows across 8 cores
(128 output rows + 8 halo rows per core); per-pixel kernels shard the
same way; no cross-core communication.

v4 design (per core, W processed in two 512-wide halves):
- All device inputs are fp16, cast and laid out on the HOST, so DMA
  streams exactly the bytes the engines read (no cast-DMA f32 reads,
  no on-chip repack):
  - the image arrives as one [128, 9*3*520] tile per half: partition p
    holds, for each (ki, c), the 520-wide padded row p+ki. 9x row
    duplication is baked in on the host (tap-row shifts cross partition
    bases, which compute engines cannot do).
  - the weights arrive as 9 per-ki tap-plane tiles [128, 9*520]: block
    kj holds wt[., w, ki, kj] pre-shifted by kj (x = w + kj) and
    zero-padded, so every DVE operand starts 4B-aligned with step 1.
- DVE: ONE fp16 tensor_mul per (half, ki) covering all 3 channels and
  all 9 kj taps (free dims [(c,3),(kj,9),(x,520)]; the image operand
  broadcasts over kj with a 0-stride dim) -> 2x_1P mode throughout.
- PE accumulates the 81 taps per channel into PSUM with
  identity-stationary matmuls; the kj realignment happens for free via
  the moving-operand offset (window x in [kj, kj+512)).
- ACT evacuates PSUM to a planar staging tile and issues the output
  store; all loads go on the sync HWDGE ring (SWDGE would contend with
  DVE's 2-port mode for SBUF).

Projected per-core busy: DVE ~133us (bottleneck), PE ~105us, DMA ~30MB
~85us, ACT ~4us.
"""

import numpy as np

import concourse.bass as bass
import concourse.bacc as bacc
import concourse.mybir as mybir
from concourse.bass import AP
from concourse.bass_utils import run_bass_kernel_spmd
from concourse.masks import make_identity
from concourse.tile import TileContext

K = 9
PAD = K // 2  # 4
H = 1024
W = 1024
C = 3
NCORES = 8
R = H // NCORES  # 128 rows per core
HALF = W // 2  # 512
BW = HALF + 2 * PAD  # 520: per-tap plane width (kj shift baked in)
FKI = K * BW  # 4680: free size of one per-ki weight tile
FIMG = K * C * BW  # 14040: free size of one per-half image tile

f32 = mybir.dt.float32
f16 = mybir.dt.float16
f8 = mybir.dt.float8e4

last_results = None  # stash for test harness introspection


def _sub_ap(base: AP, free_off: int, dims) -> AP:
    """Build a free-dim access pattern on `base` (a full-tile [P, F] AP):
    keep the partition dim, replace free dims with `dims` ([step, count]
    pairs, in elements) at element offset `free_off`."""
    ap_pairs = [list(p) for p in base.ap]
    part = ap_pairs[0]
    return AP(
        base.tensor,
        base.offset + free_off,
        [part] + [[int(s), int(n)] for s, n in dims],
    )


def build_v4(rows=R, trn="TRN2", n_reps=1, probe=0, pair_kj=False, ident_f8=False,
             dedup_ldw=True):
    """Per-core program. img [rows, 2*FIMG] f16, wt [rows, 2*K*FKI] f16,
    out [rows, 2*C*HALF] f32 (half-major, then channel-planar).

    probe: timing probes. 1 = skip PE/evac/store (DVE+DMA only);
    2 = skip DVE products (PE+DMA only).
    pair_kj: one N=1024 matmul per kj pair with a 0-stride PSUM out AP —
    REJECTED by walrus codegen ISA checks; kept for reference.
    ident_f8: fp8e4 identity stationary (exact 0/1, faster FWL load).
    dedup_ldw: the Tile scheduler emits one LDWEIGHTS per matmul; with an
    identical full-array stationary each reload serializes after the
    previous matmul's stream (row_grp conflict), costing ~53ns x 486 MMs.
    This pass drops wait-free LDWEIGHTS that reload the stationary the PE
    already holds (measured 266 -> ~216 ns/matmul)."""
    nc = bacc.Bacc(trn)
    img = nc.declare_dram_parameter("img", [rows, 2 * FIMG], f16, isOutput=False)
    wt = nc.declare_dram_parameter("wt", [rows, 2 * K * FKI], f16, isOutput=False)
    out = nc.declare_dram_parameter("out", [rows, 2 * C * HALF], f32, isOutput=True)

    with TileContext(nc) as tc:
        with (
            tc.tile_pool(name="singles", bufs=1) as singles,
            tc.tile_pool(name="imgp", bufs=12) as imgp,
            tc.tile_pool(name="wtp", bufs=6) as wtp,
            tc.tile_pool(name="prodp", bufs=3) as prodp,
            tc.tile_pool(name="outp", bufs=2) as outp,
            tc.tile_pool(name="psump", bufs=2, space="PSUM") as psump,
        ):
            ident = singles.tile([128, 128], f8 if ident_f8 else f16)
            make_identity(nc, ident[:])
            fake = None
            if probe in (2, 6):  # decoupling probes: PE streams this
                fake = singles.tile([rows, C * FKI], f16)
                nc.vector.memset(fake[:], 0.5)

            def _do_half(half):
                # per-ki image slices and weight planes, interleaved on the
                # sync ring so the first products start after ~1.6MB lands
                ims = {}
                wts = {}
                for ki in range(K):
                    wt_t = wtp.tile([rows, FKI], f16, tag="wt")
                    col0 = (half * K + ki) * FKI
                    nc.sync.dma_start(out=wt_t[:], in_=wt[:, col0 : col0 + FKI])
                    wts[ki] = wt_t
                    im_t = imgp.tile([rows, C * BW], f16, tag="im")
                    icol0 = (half * K + ki) * C * BW
                    nc.sync.dma_start(
                        out=im_t[:], in_=img[:, icol0 : icol0 + C * BW]
                    )
                    ims[ki] = im_t

                ps = {}
                for c in range(C):
                    ps_t = psump.tile([rows, HALF], f32, tag=f"ps{c}")
                    ps[c] = ps_t

                for ki in range(K):
                    if probe == 2:
                        prod = fake  # PE-only probe: stream a static tile
                    else:
                        prod = prodp.tile([rows, C * FKI], f16, tag="prod")
                    # prod[c, kj, x] = img[p+ki, x, c] * wt_shift[kj][x]
                    if probe != 2:
                        nc.vector.tensor_mul(
                            _sub_ap(prod[:], 0, [[FKI, C], [BW, K], [1, BW]]),
                            _sub_ap(ims[ki][:], 0, [[BW, C], [0, K], [1, BW]]),
                            _sub_ap(wts[ki][:], 0, [[0, C], [BW, K], [1, BW]]),
                        )
                    if probe == 1:
                        continue
                    if probe == 6:
                        prod = fake
                    for c in range(C):
                        for kj in range(K):
                            # window x in [kj, kj+512) realigns tap kj onto w
                            nc.tensor.matmul(
                                ps[c][:],
                                ident[:rows, :rows],
                                _sub_ap(
                                    prod[:], c * FKI + kj * BW + kj, [[1, HALF]]
                                ),
                                start=(ki == 0 and kj == 0),
                                stop=(ki == K - 1 and kj == K - 1),
                            )

                if probe != 1:
                    ostage = outp.tile([rows, C * HALF], f32, tag="ostage")
                    for c in range(C):
                        nc.scalar.copy(
                            out=ostage[:, c * HALF : (c + 1) * HALF],
                            in_=ps[c][:],
                        )
                    nc.scalar.dma_start(
                        out=out[:, half * C * HALF : (half + 1) * C * HALF],
                        in_=ostage[:],
                    )

            def _body():
                for half in range(2):
                    _do_half(half)

            if n_reps == 1:
                _body()
            elif n_reps < 0:  # unrolled, for the timeline simulator
                for _ in range(-n_reps):
                    _body()
            else:
                with tc.For_i(0, n_reps, 1):
                    _body()

    if dedup_ldw:
        # Runs after the Tile scheduler (which created the per-matmul
        # LDWEIGHTS) and before nc.compile() (which would move matmul
        # waits onto them). Only wait/update-free exact-duplicate reloads
        # within a block are dropped, so synchronization is untouched.
        for f in nc.m.functions:
            for b in f.blocks:
                prev_key = None
                keep = []
                for i in b.instructions:
                    if i.opcode == "Ldweights":
                        key = repr(i.ins[0])
                        si = i.sync_info
                        clean = not si or (not si.on_wait and not si.on_update)
                        if key == prev_key and clean:
                            continue
                        prev_key = key
                    keep.append(i)
                b.instructions[:] = keep

    nc.compile()
    return nc


def _prep_inputs(unet_out: np.ndarray, cnn_out: np.ndarray):
    padded = np.pad(
        unet_out, ((PAD, PAD), (PAD, PAD), (0, 0)), mode="reflect"
    ).astype(np.float16)
    padc = np.ascontiguousarray(padded.transpose(2, 0, 1))  # [C, H+8, W+8] f16

    # image: [p][half][ki][c][x], x = padded col half*HALF + x
    img_all = np.empty((H, 2, K, C, BW), np.float16)
    for half in range(2):
        for ki in range(K):
            img_all[:, half, ki] = padc[
                :, ki : ki + H, half * HALF : half * HALF + BW
            ].transpose(1, 0, 2)

    # weights: [p][half][ki][kj][x] with tap (ki,kj) shifted by kj and
    # zero-padded to width BW
    wt_all = np.zeros((H, 2, K, K, BW), np.float16)
    cnn4 = cnn_out.reshape(H, 2, HALF, K * K)
    for ki in range(K):
        for kj in range(K):
            wt_all[:, :, ki, kj, kj : kj + HALF] = cnn4[:, :, :, ki * K + kj]

    in_maps = []
    for i in range(NCORES):
        in_maps.append(
            {
                "img": np.ascontiguousarray(
                    img_all[i * R : (i + 1) * R].reshape(R, 2 * FIMG)
                ),
                "wt": np.ascontiguousarray(
                    wt_all[i * R : (i + 1) * R].reshape(R, 2 * K * FKI)
                ),
            }
        )
    return in_maps


_nc_cache = {}


def kernel(unet_out: np.ndarray, cnn_out: np.ndarray, _reps=1, _probe=0) -> np.ndarray:
    global last_results
    unet_out = np.asarray(unet_out, dtype=np.float32)
    cnn_out = np.asarray(cnn_out, dtype=np.float32)
    if (_reps, _probe) not in _nc_cache:
        _nc_cache[(_reps, _probe)] = build_v4(n_reps=_reps, probe=_probe)
    nc = _nc_cache[(_reps, _probe)]
    in_maps = _prep_inputs(unet_out, cnn_out)
    res = run_bass_kernel_spmd(nc, in_maps, list(range(NCORES)))
    last_results = res
    outs = [
        res.results[i]["out"].reshape(R, 2, C, HALF) for i in range(NCORES)
    ]
    full = np.concatenate(outs, axis=0)  # [H, 2, C, HALF]
    return np.ascontiguousarray(full.transpose(0, 1, 3, 2).reshape(H, W, C))
